# revision 1
# baseline (speedup 1.0000x reference)
"""Locally-connected conv (LocalLinear) Trainium2 Bass kernel.

Problem: x (B=64, Cin=64, 32, 32), weight (Cout=64, Cin=64, 32, 32, 3, 3),
bias (Cout=64, 32, 32) -> out (B=64, Cout=64, 32, 32).
out[b,o,y,x] = sum_{c,u,v} xpad[b,c,y+u-1,x+v-1] * W[o,c,y,x,u,v] + bias[o,y,x]

Sharding: spatial rows across 8 cores (core i owns output rows y in
[4i, 4i+4) -> 128 locations/core).  Per location it's an independent
64x64 matmul with contraction 576 = Cin*9.

Per-core kernel layout (all DMA tiles use the full 128 partitions):
  - taps t=3u+v are split even/odd: even taps' weights+x live on SBUF
    partitions 0-63 (PE rows 0-63), odd taps on partitions 64-127
    (PE rows 64-127) -> row-tiled matmuls, two PSUM banks per loc-pair
    (psA even taps, psB odd taps), summed on DVE at the end.
  - locations are paired in the stationary columns: cols 0-63 = weights
    of loc (yy, 2*xp), cols 64-127 = loc (yy, 2*xp+1) -> col-tiled
    matmuls (tile_position col 0/64) sharing the PE array.
  - matmul inputs are fp16 (1 cycle/row on PE vs 4 for fp32); PSUM
    accumulates fp32; bias added in fp32; output returned fp32.
"""

import numpy as np

import concourse.bacc as bacc
import concourse.mybir as mybir
import concourse.tile as tile
from concourse.bass_utils import run_bass_kernel_spmd

NCORES = 8
B = 64
CIN = 64
COUT = 64
H = 32
ROWS_PER_CORE = H // NCORES  # 4
NJ = 64        # loc-pairs per core (4 yy rows x 16 xp)
JB = 16        # loc-pairs per weight DMA block (one yy row)
OUT_G = 8      # loc-pairs per output DMA

F16 = mybir.dt.float16
F32 = mybir.dt.float32

_nc_cache = None


def _build_nc():
    from contextlib import ExitStack

    nc = bacc.Bacc("TRN2", target_bir_lowering=False)

    w_d = nc.dram_tensor("w", [128, NJ, 5, 128], F16, kind="ExternalInput")
    xs_d = nc.dram_tensor("xs", [128, 6, 35, B], F16, kind="ExternalInput")
    b_d = nc.dram_tensor("bias_p", [128, NJ], F32, kind="ExternalInput")
    o_d = nc.dram_tensor("out_p", [128, NJ, B], F32, kind="ExternalOutput")

    with tile.TileContext(nc) as tc, ExitStack() as ctx:
        xpool = ctx.enter_context(tc.tile_pool(name="xpool", bufs=1))
        wpool = ctx.enter_context(tc.tile_pool(name="wpool", bufs=3))
        bpool = ctx.enter_context(tc.tile_pool(name="bpool", bufs=1))
        opool = ctx.enter_context(tc.tile_pool(name="opool", bufs=4))
        tpool = ctx.enter_context(tc.tile_pool(name="tpool", bufs=4))
        pspool = ctx.enter_context(tc.tile_pool(name="ps", bufs=4, space="PSUM"))

        xs_sb = xpool.tile([128, 6, 35, B], F16)
        nc.sync.dma_start(xs_sb[:], xs_d[:])
        bias_sb = bpool.tile([128, NJ], F32)
        nc.sync.dma_start(bias_sb[:], b_d[:])

        for jb in range(NJ // JB):
            w_sb = wpool.tile([128, JB, 5, 128], F16)
            nc.sync.dma_start(w_sb[:], w_d[:, jb * JB:(jb + 1) * JB, :, :])
            for g in range(JB // OUT_G):
                out_sb = opool.tile([128, OUT_G, B], F32)
                for j8 in range(OUT_G):
                    jj = g * OUT_G + j8
                    j = jb * JB + jj
                    yy, xp = divmod(j, 16)
                    xA = 2 * xp
                    xB = 2 * xp + 1
                    psA = pspool.tile([128, B], F32)
                    psB = pspool.tile([128, B], F32)
                    # Two phases (stationary cols 0-63 = loc xA, then 64-127
                    # = loc xB): the sim tracks PSUM accumulation groups per
                    # bank zero-region, so the two col-groups' accumulation
                    # groups in one bank must not interleave.
                    for xloc, tp in ((xA, 0), (xB, 64)):
                        csl = slice(tp, tp + 64)
                        for th in range(5):
                            u, v = divmod(2 * th, 3)
                            nc.tensor.matmul(
                                psA[csl, :], w_sb[0:64, jj, th, csl],
                                xs_sb[0:64, yy + u, xloc + v, :],
                                start=(th == 0), stop=(th == 4),
                                tile_position=(0, tp))
                            if th < 4:
                                u2, v2 = divmod(2 * th + 1, 3)
                                nc.tensor.matmul(
                                    psB[csl, :], w_sb[64:128, jj, th, csl],
                                    xs_sb[64:128, yy + u2, xloc + v2, :],
                                    start=(th == 0), stop=(th == 3),
                                    tile_position=(64, tp))
                    # DVE can read only one PSUM operand per op, so the
                    # drain is two DVE ops; keeping both on DVE leaves
                    # ScalarE free to issue output DMAs without queueing
                    # behind slow ACT table ops.
                    tmp = tpool.tile([128, B], F32)
                    nc.vector.tensor_scalar_add(
                        tmp[:], psB[:], bias_sb[:, j:j + 1])
                    nc.vector.tensor_add(out_sb[:, j8, :], psA[:], tmp[:])
                j0 = jb * JB + g * OUT_G
                nc.scalar.dma_start(o_d[:, j0:j0 + OUT_G, :], out_sb[:])

    nc.compile()
    return nc


def get_nc():
    global _nc_cache
    if _nc_cache is None:
        _nc_cache = _build_nc()
    return _nc_cache


def prep_inputs(x, weight, bias):
    """Host-side resharding/relayout -> list of 8 per-core input dicts."""
    x = np.asarray(x, dtype=np.float32)
    weight = np.asarray(weight, dtype=np.float32)
    bias = np.asarray(bias, dtype=np.float32)

    # x slices with halo, padded: xs[i, p, r, cx, b]
    #   p<64: c = p (even taps), p>=64: c = p-64 (odd taps), same data.
    #   local row r in [0,6) = global y 4i-1+r; window col cx = global x-1+cx
    xp_ = np.zeros((B, CIN, H + 2, H + 2), np.float32)
    xp_[:, :, 1:H + 1, 1:H + 1] = x
    xs = np.zeros((NCORES, 128, 6, 35, B), np.float16)
    for i in range(NCORES):
        s = xp_[:, :, 4 * i:4 * i + 6, :].transpose(1, 2, 3, 0)  # (c,6,34,b)
        xs[i, 0:64, :, 0:34, :] = s
        xs[i, 64:128, :, 0:34, :] = s

    # weights: wp[i, p=(pe,c), j=(yy,xp), th, oo=(xe,o)], tap t = 2*th+pe
    W10 = np.zeros((COUT, CIN, H, H, 10), np.float32)
    W10[..., :9] = weight.reshape(COUT, CIN, H, H, 9)
    A = W10.reshape(COUT, CIN, NCORES, 4, 16, 2, 5, 2)  # o c i yy xp xe th pe
    wp = A.transpose(2, 7, 1, 3, 4, 6, 5, 0).reshape(NCORES, 128, NJ, 5, 128)
    wp = np.ascontiguousarray(wp, dtype=np.float16)

    # bias: bp[i, oo=(xe,o), j=(yy,xp)]
    Bb = bias.reshape(COUT, NCORES, 4, 16, 2)  # o i yy xp xe
    bp = np.ascontiguousarray(
        Bb.transpose(1, 4, 0, 2, 3).reshape(NCORES, 128, NJ), dtype=np.float32)

    return [
        {"w": np.ascontiguousarray(wp[i]),
         "xs": np.ascontiguousarray(xs[i]),
         "bias_p": bp[i]}
        for i in range(NCORES)
    ]


def unpack_output(results):
    """results: list of 8 dicts with 'out_p' [128, NJ, B] -> (B, COUT, H, H)."""
    allout = np.stack([r["out_p"] for r in results])  # (8, 128, 64, 64)
    a = allout.reshape(NCORES, 2, COUT, 4, 16, B)     # i xe o yy xp b
    out = a.transpose(5, 2, 0, 3, 4, 1).reshape(B, COUT, H, H)
    return np.ascontiguousarray(out, dtype=np.float32)


def kernel(x, weight, bias, _trace=False, _tmpdir=None):
    nc = get_nc()
    in_maps = prep_inputs(x, weight, bias)
    res = run_bass_kernel_spmd(
        nc, in_maps, core_ids=list(range(NCORES)),
        trace=_trace, tmpdir=_tmpdir,
        **({"trace_cores": list(range(NCORES))} if _trace else {}),
    )
    out = unpack_output(res.results)
    if _trace:
        kernel.last_results = res
    return out

